# revision 10
# baseline (speedup 1.0000x reference)
"""Trainium2 Bass kernel for nn_ButterflyLayer1D.

Data-parallel across 8 NeuronCores: each core processes 128 of the 1024
samples; the butterfly filter tree is replicated to every core.

Per-core layout convention: activations live in SBUF as
(channels=128 partitions, free = [branch..., position..., sample(128)])
with samples innermost, so every matmul is a K=128 x M=128 weight applied
to 512-column tiles of the 8192-column activation plane.  All nine stages
(input conv, 3 down levels, middle switch, 3 up levels, output conv)
output exactly 8192 columns x 128 channels per core.

Matmuls run in bf16 (weights and activations; full-rate 1 col/cycle on the
PE array) with fp32 PSUM accumulation.  Per-branch biases are applied by
the Scalar/Vector engine epilogues (relu + bias from PSUM, two 1024-col
ops per psum tile on opposite engines).  The middle switch has a distinct
bias per 128-col block, which is instead seeded into PSUM by a K=4
indicator matmul before the per-(itk,itx) c-x-c matmuls accumulate on top.
"""

import sys

for _p in ("/opt/trn_rl_repo",):
    if _p not in sys.path:
        sys.path.insert(0, _p)

import numpy as np
import ml_dtypes

import concourse.bass as bass
import concourse.bacc as bacc
import concourse.mybir as mybir
from concourse.tile import TileContext
from concourse.bass_utils import run_bass_kernel_spmd

C = 128            # channels == partitions == contraction size
N_CORES = 8
NPC = 128          # samples per core
NCOL = 64 * NPC    # 8192 free columns per stage
F32 = mybir.dt.float32
BF16 = mybir.dt.bfloat16
AF = mybir.ActivationFunctionType
ALU = mybir.AluOpType

PT = 2048          # psum tile columns (4 banks); 2 tiles fill PSUM
SUB = 512          # matmul moving-operand columns
EP = 1024          # epilogue op columns (2 per psum tile, opposite engines)


def build_nc():
    nc = bacc.Bacc()

    dp = lambda name, shape, dt=BF16: nc.declare_dram_parameter(name, list(shape), dt, False)
    xt_d = dp("xt", (C, NCOL))
    wxf_d = dp("wxf", (C, C))
    w1_d = dp("w1", (C, 4 * C))
    w2_d = dp("w2", (C, 8 * C))
    w3_d = dp("w3", (C, 16 * C))
    wm_d = dp("wm", (C, 64 * C))
    w4_d = dp("w4", (C, 16 * C))
    w5_d = dp("w5", (C, 8 * C))
    w6_d = dp("w6", (C, 4 * C))
    wkf_d = dp("wkf", (C, C))
    mb2_d = dp("mb2", (4, 16 * C))
    ind_d = dp("ind", (4, 512))
    xb_d = dp("xb", (C, 1), F32)
    b1_d = dp("b1", (C, 2), F32)
    b2_d = dp("b2", (C, 4), F32)
    b3_d = dp("b3", (C, 8), F32)
    b4_d = dp("b4", (C, 8), F32)
    b5_d = dp("b5", (C, 4), F32)
    b6_d = dp("b6", (C, 2), F32)
    out_d = nc.declare_dram_parameter("out", [C, NCOL], BF16, True)

    from contextlib import ExitStack

    with TileContext(nc) as tc, ExitStack() as ctx:
        singles = ctx.enter_context(tc.tile_pool(name="weights", bufs=1))
        act_pool = ctx.enter_context(tc.tile_pool(name="act", bufs=2))
        psum_pool = ctx.enter_context(tc.tile_pool(name="psum", bufs=2, space="PSUM"))

        def load(dram, shape, dt=BF16, split=1, name=None):
            t = singles.tile(list(shape), dt, tag=name, name=name)
            step = shape[1] // split
            for i in range(split):
                nc.sync.dma_start(
                    out=t[:, i * step : (i + 1) * step],
                    in_=dram[:, i * step : (i + 1) * step],
                )
            return t

        # x first, in fine chunks so stage 0 can start on the first 512 cols
        xt = load(xt_d, (C, NCOL), split=16, name="xt_sb")
        wxf = load(wxf_d, (C, C), name="wxf_sb")
        xb = load(xb_d, (C, 1), dt=F32, name="xb_sb")
        w1 = load(w1_d, (C, 4 * C), name="w1_sb")
        b1 = load(b1_d, (C, 2), dt=F32, name="b1_sb")
        w2 = load(w2_d, (C, 8 * C), name="w2_sb")
        b2 = load(b2_d, (C, 4), dt=F32, name="b2_sb")
        w3 = load(w3_d, (C, 16 * C), name="w3_sb")
        b3 = load(b3_d, (C, 8), dt=F32, name="b3_sb")
        wm = load(wm_d, (C, 64 * C), split=4, name="wm_sb")
        mb2 = load(mb2_d, (4, 16 * C), name="mb2_sb")
        ind = load(ind_d, (4, 512), name="ind_sb")
        w4 = load(w4_d, (C, 16 * C), name="w4_sb")
        b4 = load(b4_d, (C, 8), dt=F32, name="b4_sb")
        w5 = load(w5_d, (C, 8 * C), name="w5_sb")
        b5 = load(b5_d, (C, 4), dt=F32, name="b5_sb")
        w6 = load(w6_d, (C, 4 * C), name="w6_sb")
        b6 = load(b6_d, (C, 2), dt=F32, name="b6_sb")
        wkf = load(wkf_d, (C, C), name="wkf_sb")

        ecount = [0]

        def epi(out_ap, in_ap, bias_ap, relu=True):
            """One epilogue op: out = relu(in + bias) (or copy), alternating engines."""
            ecount[0] += 1
            eng = "s" if ecount[0] % 2 else "v"
            if bias_ap is None and not relu:
                if eng == "s":
                    nc.scalar.activation(out_ap, in_ap, AF.Copy)
                else:
                    nc.vector.tensor_copy(out_ap, in_ap)
            elif bias_ap is None:
                if eng == "s":
                    nc.scalar.activation(out_ap, in_ap, AF.Relu)
                else:
                    nc.vector.tensor_scalar_max(out_ap, in_ap, 0.0)
            else:
                if eng == "s":
                    nc.scalar.activation(out_ap, in_ap, AF.Relu, bias=bias_ap)
                else:
                    nc.vector.tensor_scalar(out_ap, in_ap, bias_ap, 0.0, ALU.add, ALU.max)

        # ---------------- stage 0: input conv ----------------
        v0 = act_pool.tile([C, NCOL], BF16, tag="act", name="v0")
        for t in range(4):
            pt = psum_pool.tile([C, PT], F32, tag="pt", name="p0")
            for s in range(4):
                col = t * PT + s * SUB
                nc.tensor.matmul(
                    pt[:, s * SUB : (s + 1) * SUB],
                    wxf[:, :],
                    xt[:, col : col + SUB],
                    start=True,
                    stop=True,
                )
            for h in range(2):
                epi(
                    v0[:, t * PT + h * EP : t * PT + (h + 1) * EP],
                    pt[:, h * EP : (h + 1) * EP],
                    xb[:, 0:1],
                )

        # ---------------- down levels 1..3 ----------------
        def down_level(vin, vout, w_sb, b_sb, nb_out, l_out):
            """vin: (c, [nb_in, 2*l_out, n]); vout: (c, [nb_out, l_out, n])."""
            wv = w_sb.rearrange("p (b k d) -> p b k d", b=nb_out, k=2, d=C)
            vi = vin.rearrange("p (b l k n) -> p b l k n", b=nb_out // 2, l=l_out, k=2, n=NPC)
            vo = vout.rearrange("p (b l n) -> p b l n", b=nb_out, l=l_out, n=NPC)
            cpb = l_out * NPC  # columns per output branch (>= 1024 for levels 1..3)
            for t in range(NCOL // PT):
                pt = psum_pool.tile([C, PT], F32, tag="pt", name="pd")
                for s in range(PT // SUB):
                    col = t * PT + s * SUB
                    b = col // cpb
                    l0 = (col % cpb) // NPC
                    for k in range(2):
                        nc.tensor.matmul(
                            pt[:, s * SUB : (s + 1) * SUB],
                            wv[:, b, k, :],
                            vi[:, b // 2, l0 : l0 + SUB // NPC, k, :],
                            start=(k == 0),
                            stop=(k == 1),
                        )
                for h in range(2):
                    col = t * PT + h * EP
                    b = col // cpb
                    l0 = (col % cpb) // NPC
                    epi(
                        vo[:, b, l0 : l0 + EP // NPC, :],
                        pt[:, h * EP : (h + 1) * EP],
                        b_sb[:, b : b + 1],
                    )

        v1 = act_pool.tile([C, NCOL], BF16, tag="act", name="v1")
        down_level(v0, v1, w1, b1, 2, 32)
        v2 = act_pool.tile([C, NCOL], BF16, tag="act", name="v2")
        down_level(v1, v2, w2, b2, 4, 16)
        v3 = act_pool.tile([C, NCOL], BF16, tag="act", name="v3")
        down_level(v2, v3, w3, b3, 8, 8)

        # ---------------- middle switch ----------------
        # v3: (c, [itk=8, itx=8, n]); vm: (c, [itx=8, itk=8, n])
        # Per-(itx,itk) bias seeded into PSUM by a K=4 indicator matmul,
        # then the per-block c x c matmuls accumulate on top.
        vm = act_pool.tile([C, NCOL], BF16, tag="act", name="vm")
        v3v = v3.rearrange("p (k x n) -> p k x n", k=8, x=8, n=NPC)
        wmv = wm.rearrange("p (k x d) -> p k x d", k=8, x=8, d=C)
        for t in range(4):  # tile t covers itx in {2t, 2t+1}
            pt = psum_pool.tile([C, PT], F32, tag="pt", name="pm")
            for s in range(4):  # 512-col subtile = 4 blocks of 128
                u = 4 * t + s
                nc.tensor.matmul(
                    pt[:, s * SUB : (s + 1) * SUB],
                    mb2[:, u * C : (u + 1) * C],
                    ind[:, :],
                    start=True,
                    stop=False,
                    skip_group_check=True,
                )
                for bi in range(4):
                    blk = 4 * s + bi  # block within tile; global = 16t + blk
                    xl, kk = blk // 8, blk % 8
                    nc.tensor.matmul(
                        pt[:, blk * NPC : (blk + 1) * NPC],
                        wmv[:, kk, 2 * t + xl, :],
                        v3v[:, kk, 2 * t + xl, :],
                        start=False,
                        stop=(bi == 3),
                        skip_group_check=True,
                    )
            for h in range(2):
                epi(
                    vm[:, t * PT + h * EP : t * PT + (h + 1) * EP],
                    pt[:, h * EP : (h + 1) * EP],
                    None,
                )

        # ---------------- up levels 4..6 ----------------
        def up_level(vin, vout, w_sb, b_sb, nb_in, l_in):
            """vin: (c, [x=nb_in, l_in, n]); vout: (c, [xo=nb_in/2, 2*l_in, n]);
            vout[:, xo, 2*l+j, :] = relu(sum_k vin[:, 2xo+k, l, :] @ W[xo,j,k] + B[xo,j])."""
            nbo = nb_in // 2
            wv = w_sb.rearrange("p (x j k d) -> p x j k d", x=nbo, j=2, k=2, d=C)
            vi = vin.rearrange("p (x l n) -> p x l n", x=nb_in, l=l_in, n=NPC)
            vo = vout.rearrange("p (x l j n) -> p x l j n", x=nbo, l=l_in, j=2, n=NPC)
            cpb = l_in * NPC  # columns per (xo, j) output block
            for t in range(NCOL // PT):
                pt = psum_pool.tile([C, PT], F32, tag="pt", name="pu")
                for s in range(PT // SUB):
                    col = t * PT + s * SUB
                    g = col // cpb          # global (xo, j) block index, j-minor
                    xo, j = g // 2, g % 2
                    lt0 = (col % cpb) // NPC
                    for k in range(2):
                        nc.tensor.matmul(
                            pt[:, s * SUB : (s + 1) * SUB],
                            wv[:, xo, j, k, :],
                            vi[:, 2 * xo + k, lt0 : lt0 + SUB // NPC, :],
                            start=(k == 0),
                            stop=(k == 1),
                        )
                for h in range(2):
                    col = t * PT + h * EP
                    g = col // cpb
                    xo, j = g // 2, g % 2
                    lt0 = (col % cpb) // NPC
                    epi(
                        vo[:, xo, lt0 : lt0 + EP // NPC, j, :],
                        pt[:, h * EP : (h + 1) * EP],
                        b_sb[:, 2 * xo + j : 2 * xo + j + 1],
                    )

        v4 = act_pool.tile([C, NCOL], BF16, tag="act", name="v4")
        up_level(vm, v4, w4, b4, 8, 8)
        v5 = act_pool.tile([C, NCOL], BF16, tag="act", name="v5")
        up_level(v4, v5, w5, b5, 4, 16)
        v6 = act_pool.tile([C, NCOL], BF16, tag="act", name="v6")
        up_level(v5, v6, w6, b6, 2, 32)

        # ---------------- output conv (no bias / relu) ----------------
        yo = act_pool.tile([C, NCOL], BF16, tag="act", name="yo")
        for t in range(4):
            pt = psum_pool.tile([C, PT], F32, tag="pt", name="po")
            for s in range(4):
                col = t * PT + s * SUB
                nc.tensor.matmul(
                    pt[:, s * SUB : (s + 1) * SUB],
                    wkf[:, :],
                    v6[:, col : col + SUB],
                    start=True,
                    stop=True,
                )
            for h in range(2):
                epi(
                    yo[:, t * PT + h * EP : t * PT + (h + 1) * EP],
                    pt[:, h * EP : (h + 1) * EP],
                    None,
                    relu=False,
                )
                nc.sync.dma_start(
                    out=out_d[:, t * PT + h * EP : t * PT + (h + 1) * EP],
                    in_=yo[:, t * PT + h * EP : t * PT + (h + 1) * EP],
                )

    nc.finalize()
    return nc


_NC_CACHE = {}


def _get_nc():
    if "nc" not in _NC_CACHE:
        _NC_CACHE["nc"] = build_nc()
    return _NC_CACHE["nc"]


def _prep_in_maps(inputs):
    x = np.asarray(inputs["x"], np.float32)
    bf = lambda a: np.ascontiguousarray(np.asarray(a, np.float32)).astype(ml_dtypes.bfloat16)
    f32 = lambda a: np.ascontiguousarray(np.asarray(a, np.float32))
    mb = np.asarray(inputs["mb"], np.float32)  # (k=8, x=8, c)
    mbg = mb.transpose(1, 0, 2).reshape(64, C)  # row g = x*8 + k
    mb2 = mbg.reshape(16, 4, C).transpose(1, 0, 2).reshape(4, 16 * C)
    ind = np.zeros((4, 512), np.float32)
    for ki in range(4):
        ind[ki, ki * NPC : (ki + 1) * NPC] = 1.0
    shared = {
        "wxf": bf(inputs["xf"]),  # (f=128, c) as lhsT directly
        "w1": bf(np.asarray(inputs["f1"], np.float32).reshape(4, C, C).transpose(1, 0, 2).reshape(C, 4 * C)),
        "w2": bf(np.asarray(inputs["f2"], np.float32).reshape(8, C, C).transpose(1, 0, 2).reshape(C, 8 * C)),
        "w3": bf(np.asarray(inputs["f3"], np.float32).reshape(16, C, C).transpose(1, 0, 2).reshape(C, 16 * C)),
        "wm": bf(np.asarray(inputs["md"], np.float32).reshape(64, C, C).transpose(1, 0, 2).reshape(C, 64 * C)),
        "w4": bf(np.asarray(inputs["f4"], np.float32).reshape(16, C, C).transpose(1, 0, 2).reshape(C, 16 * C)),
        "w5": bf(np.asarray(inputs["f5"], np.float32).reshape(8, C, C).transpose(1, 0, 2).reshape(C, 8 * C)),
        "w6": bf(np.asarray(inputs["f6"], np.float32).reshape(4, C, C).transpose(1, 0, 2).reshape(C, 4 * C)),
        "wkf": bf(inputs["kf"]),  # (c, kout) as lhsT directly
        "mb2": bf(mb2),
        "ind": bf(ind),
        "xb": f32(np.asarray(inputs["xb"]).reshape(C, 1)),
        "b1": f32(np.asarray(inputs["b1"]).T),
        "b2": f32(np.asarray(inputs["b2"]).T),
        "b3": f32(np.asarray(inputs["b3"]).T),
        "b4": f32(np.asarray(inputs["b4"]).T),
        "b5": f32(np.asarray(inputs["b5"]).T),
        "b6": f32(np.asarray(inputs["b6"]).T),
    }
    in_maps = []
    for i in range(N_CORES):
        xs = x[i * NPC : (i + 1) * NPC]  # (128, 8192)
        xt = (
            np.ascontiguousarray(xs.reshape(NPC, 64, C).transpose(2, 1, 0))
            .reshape(C, NCOL)
            .astype(ml_dtypes.bfloat16)
        )
        in_maps.append({"xt": xt, **shared})
    return in_maps


def _gather(results):
    outs = []
    for i in range(N_CORES):
        r = np.asarray(results[i]["out"]).astype(np.float32)  # (C=k_out, [l=64, n=128])
        outs.append(r.reshape(C, 64, NPC).transpose(2, 1, 0).reshape(NPC, 64 * C))
    return np.concatenate(outs, axis=0).astype(np.float32)


def _enable_ntff_hook():
    """Register the axon NTFF profiling hook (missing from this image's
    antenv) so run_bass_kernel_spmd(trace=True) can measure HW exec time."""
    import types

    if "antenv.axon_hooks" in sys.modules:
        return
    import antenv
    from trn_agent_boot.trn_boot import _ntff_profile_via_ctypes

    hook = _ntff_profile_via_ctypes("/opt/axon/libaxon_pjrt.so")
    mod = types.ModuleType("antenv.axon_hooks")
    mod.get_axon_ntff_profile_hook = lambda: hook
    mod.set_axon_ntff_profile_hook = lambda h: None
    sys.modules["antenv.axon_hooks"] = mod
    antenv.axon_hooks = mod
    import concourse.bass_utils as bu

    bu.upload_artifacts = lambda tmpdir: tmpdir  # keep artifacts local


def run(inputs, trace=False, **kw):
    nc = _get_nc()
    in_maps = _prep_in_maps(inputs)
    if trace:
        _enable_ntff_hook()
    res = run_bass_kernel_spmd(nc, in_maps, core_ids=list(range(N_CORES)), trace=trace, **kw)
    return _gather(res.results), res


def kernel(**inputs) -> np.ndarray:
    out, _ = run(inputs, trace=False)
    return out


# revision 13
# speedup vs baseline: 1.3283x; 1.3283x over previous
"""Trainium2 Bass kernel for nn_ButterflyLayer1D.

Data-parallel across 8 NeuronCores: each core processes 128 of the 1024
samples; the butterfly filter tree is replicated to every core.

Per-core layout convention: activations live in SBUF as
(channels=128 partitions, free = [branch..., position..., sample(128)])
with samples innermost, so every matmul is a K=128 x M=128 weight applied
to 512-column tiles of the 8192-column activation plane.  All nine stages
(input conv, 3 down levels, middle switch, 3 up levels, output conv)
output exactly 8192 columns x 128 channels per core.

Matmuls run in bf16 (weights and activations; full-rate 1 col/cycle on the
PE array) with fp32 PSUM accumulation.  Per-branch biases are applied by
the Scalar/Vector engine epilogues (relu + bias from PSUM, two 1024-col
ops per psum tile on opposite engines).  The middle switch has a distinct
bias per 128-col block, which is instead seeded into PSUM by a K=4
indicator matmul before the per-(itk,itx) c-x-c matmuls accumulate on top.
"""

import sys

for _p in ("/opt/trn_rl_repo",):
    if _p not in sys.path:
        sys.path.insert(0, _p)

import numpy as np
import ml_dtypes

import concourse.bass as bass
import concourse.bacc as bacc
import concourse.mybir as mybir
from concourse.tile import TileContext
from concourse.bass_utils import run_bass_kernel_spmd

C = 128            # channels == partitions == contraction size
N_CORES = 8
NPC = 128          # samples per core
NCOL = 64 * NPC    # 8192 free columns per stage
F32 = mybir.dt.float32
BF16 = mybir.dt.bfloat16
AF = mybir.ActivationFunctionType
ALU = mybir.AluOpType

PT = 2048          # psum tile columns (4 banks); 2 tiles fill PSUM
SUB = 512          # matmul moving-operand columns
EP = 1024          # epilogue op columns (2 per psum tile, opposite engines)


def build_nc():
    nc = bacc.Bacc()

    dp = lambda name, shape, dt=BF16: nc.declare_dram_parameter(name, list(shape), dt, False)
    xt_d = dp("xt", (C, NCOL))
    wxf_d = dp("wxf", (C, C))
    w1_d = dp("w1", (C, 4 * C))
    w2_d = dp("w2", (C, 8 * C))
    w3_d = dp("w3", (C, 16 * C))
    wm_d = dp("wm", (C, 64 * C))
    w4_d = dp("w4", (C, 16 * C))
    w5_d = dp("w5", (C, 8 * C))
    w6_d = dp("w6", (C, 4 * C))
    wkf_d = dp("wkf", (C, C))
    mb_d = dp("mb", (C, 64), F32)
    xb_d = dp("xb", (C, 1), F32)
    b1_d = dp("b1", (C, 2), F32)
    b2_d = dp("b2", (C, 4), F32)
    b3_d = dp("b3", (C, 8), F32)
    b4_d = dp("b4", (C, 8), F32)
    b5_d = dp("b5", (C, 4), F32)
    b6_d = dp("b6", (C, 2), F32)
    out_d = nc.declare_dram_parameter("out", [C, NCOL], BF16, True)

    from contextlib import ExitStack

    with TileContext(nc) as tc, ExitStack() as ctx:
        singles = ctx.enter_context(tc.tile_pool(name="weights", bufs=1))
        act_pool = ctx.enter_context(tc.tile_pool(name="act", bufs=2))
        psum_pool = ctx.enter_context(tc.tile_pool(name="psum", bufs=2, space="PSUM"))

        def load(dram, shape, dt=BF16, split=1, name=None):
            t = singles.tile(list(shape), dt, tag=name, name=name)
            step = shape[1] // split
            for i in range(split):
                nc.sync.dma_start(
                    out=t[:, i * step : (i + 1) * step],
                    in_=dram[:, i * step : (i + 1) * step],
                )
            return t

        # Critical-path loads first: the first 512 columns of x plus the
        # stage-0 weights, then the rest of x in BW-friendly chunks.
        xt = singles.tile([C, NCOL], BF16, tag="xt_sb", name="xt_sb")
        nc.sync.dma_start(out=xt[:, 0:512], in_=xt_d[:, 0:512])
        wxf = load(wxf_d, (C, C), name="wxf_sb")
        xb = load(xb_d, (C, 1), dt=F32, name="xb_sb")
        for c0, c1 in ((512, 1024), (1024, 2048), (2048, 4096), (4096, 6144), (6144, 8192)):
            nc.sync.dma_start(out=xt[:, c0:c1], in_=xt_d[:, c0:c1])
        w1 = load(w1_d, (C, 4 * C), name="w1_sb")
        b1 = load(b1_d, (C, 2), dt=F32, name="b1_sb")
        w2 = load(w2_d, (C, 8 * C), name="w2_sb")
        b2 = load(b2_d, (C, 4), dt=F32, name="b2_sb")
        w3 = load(w3_d, (C, 16 * C), name="w3_sb")
        b3 = load(b3_d, (C, 8), dt=F32, name="b3_sb")
        wm = load(wm_d, (C, 64 * C), split=4, name="wm_sb")
        mb = load(mb_d, (C, 64), dt=F32, name="mb_sb")
        w4 = load(w4_d, (C, 16 * C), name="w4_sb")
        b4 = load(b4_d, (C, 8), dt=F32, name="b4_sb")
        w5 = load(w5_d, (C, 8 * C), name="w5_sb")
        b5 = load(b5_d, (C, 4), dt=F32, name="b5_sb")
        w6 = load(w6_d, (C, 4 * C), name="w6_sb")
        b6 = load(b6_d, (C, 2), dt=F32, name="b6_sb")
        wkf = load(wkf_d, (C, C), name="wkf_sb")

        load_ns = {"s": 0.0, "v": 0.0}

        def epi(out_ap, in_ap, bias_ap, relu=True, cols=EP):
            """One epilogue op: out = relu(in + bias) (or copy); greedy engine balance."""
            cost = {"s": (352 + cols) / 1.2, "v": (120 + cols) / 0.96}
            eng = "s" if load_ns["s"] + cost["s"] <= load_ns["v"] + cost["v"] else "v"
            load_ns[eng] += cost[eng]
            if bias_ap is None and not relu:
                if eng == "s":
                    nc.scalar.activation(out_ap, in_ap, AF.Copy)
                else:
                    nc.vector.tensor_copy(out_ap, in_ap)
            elif bias_ap is None:
                if eng == "s":
                    nc.scalar.activation(out_ap, in_ap, AF.Relu)
                else:
                    nc.vector.tensor_scalar_max(out_ap, in_ap, 0.0)
            else:
                if eng == "s":
                    nc.scalar.activation(out_ap, in_ap, AF.Relu, bias=bias_ap)
                else:
                    nc.vector.tensor_scalar(out_ap, in_ap, bias_ap, 0.0, ALU.add, ALU.max)

        # ---------------- stage 0: input conv ----------------
        v0 = act_pool.tile([C, NCOL], BF16, tag="act", name="v0")
        for t in range(4):
            pt = psum_pool.tile([C, PT], F32, tag="pt", name="p0")
            for s in range(4):
                col = t * PT + s * SUB
                nc.tensor.matmul(
                    pt[:, s * SUB : (s + 1) * SUB],
                    wxf[:, :],
                    xt[:, col : col + SUB],
                    start=True,
                    stop=True,
                )
            for h in range(2):
                epi(
                    v0[:, t * PT + h * EP : t * PT + (h + 1) * EP],
                    pt[:, h * EP : (h + 1) * EP],
                    xb[:, 0:1],
                )

        # ---------------- down levels 1..3 ----------------
        def down_level(vin, vout, w_sb, b_sb, nb_out, l_out):
            """vin: (c, [nb_in, 2*l_out, n]); vout: (c, [nb_out, l_out, n])."""
            wv = w_sb.rearrange("p (b k d) -> p b k d", b=nb_out, k=2, d=C)
            vi = vin.rearrange("p (b l k n) -> p b l k n", b=nb_out // 2, l=l_out, k=2, n=NPC)
            vo = vout.rearrange("p (b l n) -> p b l n", b=nb_out, l=l_out, n=NPC)
            cpb = l_out * NPC  # columns per output branch (>= 1024 for levels 1..3)
            for t in range(NCOL // PT):
                pt = psum_pool.tile([C, PT], F32, tag="pt", name="pd")
                for s in range(PT // SUB):
                    col = t * PT + s * SUB
                    b = col // cpb
                    l0 = (col % cpb) // NPC
                    for k in range(2):
                        nc.tensor.matmul(
                            pt[:, s * SUB : (s + 1) * SUB],
                            wv[:, b, k, :],
                            vi[:, b // 2, l0 : l0 + SUB // NPC, k, :],
                            start=(k == 0),
                            stop=(k == 1),
                        )
                for h in range(2):
                    col = t * PT + h * EP
                    b = col // cpb
                    l0 = (col % cpb) // NPC
                    epi(
                        vo[:, b, l0 : l0 + EP // NPC, :],
                        pt[:, h * EP : (h + 1) * EP],
                        b_sb[:, b : b + 1],
                    )

        v1 = act_pool.tile([C, NCOL], BF16, tag="act", name="v1")
        down_level(v0, v1, w1, b1, 2, 32)
        v2 = act_pool.tile([C, NCOL], BF16, tag="act", name="v2")
        down_level(v1, v2, w2, b2, 4, 16)
        v3 = act_pool.tile([C, NCOL], BF16, tag="act", name="v3")
        down_level(v2, v3, w3, b3, 8, 8)

        # ---------------- middle switch ----------------
        # v3: (c, [itk=8, itx=8, n]); vm: (c, [itx=8, itk=8, n])
        # Per-(itx,itk) bias seeded into PSUM by a K=4 indicator matmul,
        # then the per-block c x c matmuls accumulate on top.
        vm = act_pool.tile([C, NCOL], BF16, tag="act", name="vm")
        v3v = v3.rearrange("p (k x n) -> p k x n", k=8, x=8, n=NPC)
        wmv = wm.rearrange("p (k x d) -> p k x d", k=8, x=8, d=C)
        for t in range(4):  # tile t covers itx in {2t, 2t+1}
            pt = psum_pool.tile([C, PT], F32, tag="pt", name="pm")
            for blk in range(16):  # block within tile; global = 16t + blk
                xl, kk = blk // 8, blk % 8
                nc.tensor.matmul(
                    pt[:, blk * NPC : (blk + 1) * NPC],
                    wmv[:, kk, 2 * t + xl, :],
                    v3v[:, kk, 2 * t + xl, :],
                    start=True,
                    stop=True,
                )
            # bias varies per 128-col block: TT-add (V/S alternating) with a
            # broadcast bias view, then in-place relu on the idle GpSimd.
            ptv = pt.rearrange("p (b n) -> p b n", b=16, n=NPC)
            bias_v = mb[:, 16 * t : 16 * (t + 1)].unsqueeze(2).broadcast_to((C, 16, NPC))
            dst = vm[:, t * PT : (t + 1) * PT]
            dstv = dst.rearrange("p (b n) -> p b n", b=16, n=NPC)
            nc.vector.tensor_tensor(dstv, ptv, bias_v, ALU.add)
            load_ns["v"] += (120 + PT) / 0.96
            nc.scalar.activation(dst, dst, AF.Relu)
            load_ns["s"] += (352 + PT) / 1.2

        # ---------------- up levels 4..6 ----------------
        def up_level(vin, vout, w_sb, b_sb, nb_in, l_in):
            """vin: (c, [x=nb_in, l_in, n]); vout: (c, [xo=nb_in/2, 2*l_in, n]);
            vout[:, xo, 2*l+j, :] = relu(sum_k vin[:, 2xo+k, l, :] @ W[xo,j,k] + B[xo,j])."""
            nbo = nb_in // 2
            wv = w_sb.rearrange("p (x j k d) -> p x j k d", x=nbo, j=2, k=2, d=C)
            vi = vin.rearrange("p (x l n) -> p x l n", x=nb_in, l=l_in, n=NPC)
            vo = vout.rearrange("p (x l j n) -> p x l j n", x=nbo, l=l_in, j=2, n=NPC)
            cpb = l_in * NPC  # columns per (xo, j) output block
            for t in range(NCOL // PT):
                pt = psum_pool.tile([C, PT], F32, tag="pt", name="pu")
                for s in range(PT // SUB):
                    col = t * PT + s * SUB
                    g = col // cpb          # global (xo, j) block index, j-minor
                    xo, j = g // 2, g % 2
                    lt0 = (col % cpb) // NPC
                    for k in range(2):
                        nc.tensor.matmul(
                            pt[:, s * SUB : (s + 1) * SUB],
                            wv[:, xo, j, k, :],
                            vi[:, 2 * xo + k, lt0 : lt0 + SUB // NPC, :],
                            start=(k == 0),
                            stop=(k == 1),
                        )
                for h in range(2):
                    col = t * PT + h * EP
                    g = col // cpb
                    xo, j = g // 2, g % 2
                    lt0 = (col % cpb) // NPC
                    epi(
                        vo[:, xo, lt0 : lt0 + EP // NPC, j, :],
                        pt[:, h * EP : (h + 1) * EP],
                        b_sb[:, 2 * xo + j : 2 * xo + j + 1],
                    )

        v4 = act_pool.tile([C, NCOL], BF16, tag="act", name="v4")
        up_level(vm, v4, w4, b4, 8, 8)
        v5 = act_pool.tile([C, NCOL], BF16, tag="act", name="v5")
        up_level(v4, v5, w5, b5, 4, 16)
        v6 = act_pool.tile([C, NCOL], BF16, tag="act", name="v6")
        up_level(v5, v6, w6, b6, 2, 32)

        # ---------------- output conv (no bias / relu) ----------------
        yo = act_pool.tile([C, NCOL], BF16, tag="act", name="yo")
        for t in range(4):
            pt = psum_pool.tile([C, PT], F32, tag="pt", name="po")
            for s in range(4):
                col = t * PT + s * SUB
                nc.tensor.matmul(
                    pt[:, s * SUB : (s + 1) * SUB],
                    wkf[:, :],
                    v6[:, col : col + SUB],
                    start=True,
                    stop=True,
                )
            for h in range(2):
                epi(
                    yo[:, t * PT + h * EP : t * PT + (h + 1) * EP],
                    pt[:, h * EP : (h + 1) * EP],
                    None,
                    relu=False,
                )
                nc.sync.dma_start(
                    out=out_d[:, t * PT + h * EP : t * PT + (h + 1) * EP],
                    in_=yo[:, t * PT + h * EP : t * PT + (h + 1) * EP],
                )

    nc.finalize()
    return nc


_NC_CACHE = {}


def _get_nc():
    if "nc" not in _NC_CACHE:
        _NC_CACHE["nc"] = build_nc()
    return _NC_CACHE["nc"]


def _prep_in_maps(inputs):
    x = np.asarray(inputs["x"], np.float32)
    bf = lambda a: np.ascontiguousarray(np.asarray(a, np.float32)).astype(ml_dtypes.bfloat16)
    f32 = lambda a: np.ascontiguousarray(np.asarray(a, np.float32))
    mbv = np.asarray(inputs["mb"], np.float32)  # (k=8, x=8, c)
    mbT = mbv.transpose(1, 0, 2).reshape(64, C).T  # (c, 64), col = x*8 + k
    shared = {
        "wxf": bf(inputs["xf"]),  # (f=128, c) as lhsT directly
        "w1": bf(np.asarray(inputs["f1"], np.float32).reshape(4, C, C).transpose(1, 0, 2).reshape(C, 4 * C)),
        "w2": bf(np.asarray(inputs["f2"], np.float32).reshape(8, C, C).transpose(1, 0, 2).reshape(C, 8 * C)),
        "w3": bf(np.asarray(inputs["f3"], np.float32).reshape(16, C, C).transpose(1, 0, 2).reshape(C, 16 * C)),
        "wm": bf(np.asarray(inputs["md"], np.float32).reshape(64, C, C).transpose(1, 0, 2).reshape(C, 64 * C)),
        "w4": bf(np.asarray(inputs["f4"], np.float32).reshape(16, C, C).transpose(1, 0, 2).reshape(C, 16 * C)),
        "w5": bf(np.asarray(inputs["f5"], np.float32).reshape(8, C, C).transpose(1, 0, 2).reshape(C, 8 * C)),
        "w6": bf(np.asarray(inputs["f6"], np.float32).reshape(4, C, C).transpose(1, 0, 2).reshape(C, 4 * C)),
        "wkf": bf(inputs["kf"]),  # (c, kout) as lhsT directly
        "mb": f32(mbT),
        "xb": f32(np.asarray(inputs["xb"]).reshape(C, 1)),
        "b1": f32(np.asarray(inputs["b1"]).T),
        "b2": f32(np.asarray(inputs["b2"]).T),
        "b3": f32(np.asarray(inputs["b3"]).T),
        "b4": f32(np.asarray(inputs["b4"]).T),
        "b5": f32(np.asarray(inputs["b5"]).T),
        "b6": f32(np.asarray(inputs["b6"]).T),
    }
    in_maps = []
    for i in range(N_CORES):
        xs = x[i * NPC : (i + 1) * NPC]  # (128, 8192)
        xt = (
            np.ascontiguousarray(xs.reshape(NPC, 64, C).transpose(2, 1, 0))
            .reshape(C, NCOL)
            .astype(ml_dtypes.bfloat16)
        )
        in_maps.append({"xt": xt, **shared})
    return in_maps


def _gather(results):
    outs = []
    for i in range(N_CORES):
        r = np.asarray(results[i]["out"]).astype(np.float32)  # (C=k_out, [l=64, n=128])
        outs.append(r.reshape(C, 64, NPC).transpose(2, 1, 0).reshape(NPC, 64 * C))
    return np.concatenate(outs, axis=0).astype(np.float32)


def _enable_ntff_hook():
    """Register the axon NTFF profiling hook (missing from this image's
    antenv) so run_bass_kernel_spmd(trace=True) can measure HW exec time."""
    import types

    if "antenv.axon_hooks" in sys.modules:
        return
    import antenv
    from trn_agent_boot.trn_boot import _ntff_profile_via_ctypes

    hook = _ntff_profile_via_ctypes("/opt/axon/libaxon_pjrt.so")
    mod = types.ModuleType("antenv.axon_hooks")
    mod.get_axon_ntff_profile_hook = lambda: hook
    mod.set_axon_ntff_profile_hook = lambda h: None
    sys.modules["antenv.axon_hooks"] = mod
    antenv.axon_hooks = mod
    import concourse.bass_utils as bu

    bu.upload_artifacts = lambda tmpdir: tmpdir  # keep artifacts local


def run(inputs, trace=False, **kw):
    nc = _get_nc()
    in_maps = _prep_in_maps(inputs)
    if trace:
        _enable_ntff_hook()
    res = run_bass_kernel_spmd(nc, in_maps, core_ids=list(range(N_CORES)), trace=trace, **kw)
    return _gather(res.results), res


def kernel(**inputs) -> np.ndarray:
    out, _ = run(inputs, trace=False)
    return out
